# revision 7
# baseline (speedup 1.0000x reference)
"""AttentionBlock (GroupNorm + single-head 4096x4096 attention + proj + residual)
on 8 Trainium2 NeuronCores.

Sharding: core c = 2*b + h handles image b (of 4), query-half h (of 2).
Each core:
  - computes GroupNorm stats for its image (all 4096 positions),
  - computes kT [512,4096] and v [4096,512] for the full image,
  - computes qT for its 2048 query rows,
  - flash-style attention over its 2048 queries (softmax without max
    subtraction -- scores are O(6) so exp is safe in fp32),
  - projection + bias + residual for its rows.
No collectives; k/v compute is duplicated across the half-pair (~10% FLOPs).

Precision: float32r (full-speed PE, ~1e-4 rel err) for transposes, QKV,
scores and projection; bf16 for the exp(P)@V contraction; fp32 for GN stats,
softmax normalization and the residual add.
"""

import sys

sys.path.insert(0, "/opt/trn_rl_repo")

import numpy as np  # noqa: E402

import bass_rust  # noqa: E402
import concourse.bass as bass  # noqa: E402
import concourse.mybir as mybir  # noqa: E402
import concourse.tile as tile  # noqa: E402
from concourse.vector_clock import ScopedClock  # noqa: E402
from concourse.bass_utils import run_bass_kernel_spmd  # noqa: E402

F32 = mybir.dt.float32
F32R = mybir.dt.float32r
BF16 = mybir.dt.bfloat16
AF = mybir.ActivationFunctionType
OP = mybir.AluOpType
AX = mybir.AxisListType

B, H, W, C = 4, 64, 64, 512
HW = H * W            # 4096 positions per image
HALF = HW // 2        # 2048 query rows per core
GROUPS = 32
GSIZE = C // GROUPS   # 16 channels per group
EPS = 1e-5
SM_SCALE = 1.0 / float(np.sqrt(C))
N_CORES = 8
CT = C // 128         # 4 channel partition-tiles
JT = HW // 128        # 32 position partition-tiles
JC = HW // 512        # 8 position chunks (kT/v build)
QC = HALF // 512      # 4 query chunks (qT build)
IB = HALF // 512      # 4 query i-blocks (attention)


# --- workaround: walrus in this container rejects instructions carrying more
# than one sync-wait command.  Split extra waits onto same-engine NOPs placed
# immediately before the instruction (engine program order makes this exact).
def _split_multi_waits(nc, max_waits=1):
    n = 0
    for f in nc.m.functions:
        for bb in f.blocks:
            newlist = []
            for inst in bb.instructions:
                si = inst.sync_info
                waits = list(si.on_wait) if si is not None else []
                if len(waits) > max_waits:
                    n += 1
                    for k, wt in enumerate(waits[:-max_waits]):
                        nop = bass_rust.InstNoOp(
                            name=f"{inst.name}-sw{k}", engine=inst.engine)
                        nop.sync_info = mybir.SyncInfo(on_wait=[wt], on_update=[])
                        newlist.append(nop)
                    inst.sync_info = mybir.SyncInfo(
                        on_wait=waits[-max_waits:], on_update=list(si.on_update))
                newlist.append(inst)
            bb.instructions[:] = newlist
    return n


def _split_drain_and_barrier(self, tick_clock, wait_clock):
    # same as TileContext._drain_and_barrier but with the tail drain's waits
    # split onto single-wait NOPs (same walrus limitation as above).
    drain_inst = self.nc.sync.drain()
    wait_clock.add_sem_waits(
        drain_inst.ins, ScopedClock({None: tick_clock.global_clock}))
    mi = drain_inst.ins
    waits = list(mi.sync_info.on_wait) if mi.sync_info is not None else []
    if len(waits) > 1:
        mi.sync_info.on_wait = []
        for wt in waits:
            wi = self.nc.sync.nop(nofuse=True, hint="tail_drain_wait")
            wi.ins.sync_info = mybir.SyncInfo(on_wait=[wt], on_update=[])
    self.nc.all_engine_barrier()
    assert self.sems is not None
    popped = self.nc._tile_sem_poison_stack.pop()
    assert popped is self._sem_poison
    self.nc.clear_and_free_semaphores(list(self.sems.allocated().values()))
    self.nc.all_engine_barrier()


tile.TileContext._drain_and_barrier = _split_drain_and_barrier


def build_program():
    nc = bass.Bass()

    xb = nc.dram_tensor("xb", [HW, C], F32, kind="ExternalInput")
    xq = nc.dram_tensor("xq", [HALF, C], F32, kind="ExternalInput")
    wq = nc.dram_tensor("wq", [C, C], F32, kind="ExternalInput")
    wk = nc.dram_tensor("wk", [C, C], F32, kind="ExternalInput")
    wv = nc.dram_tensor("wv", [C, C], F32, kind="ExternalInput")
    wp = nc.dram_tensor("wp", [C, C], F32, kind="ExternalInput")
    bqd = nc.dram_tensor("bq", [C, 1], F32, kind="ExternalInput")
    bkd = nc.dram_tensor("bk", [C, 1], F32, kind="ExternalInput")
    bvd = nc.dram_tensor("bv", [C], F32, kind="ExternalInput")
    bpd = nc.dram_tensor("bp", [C], F32, kind="ExternalInput")
    gamd = nc.dram_tensor("gamma", [C, 1], F32, kind="ExternalInput")
    betd = nc.dram_tensor("beta", [C, 1], F32, kind="ExternalInput")
    idnd = nc.dram_tensor("idn", [128, 128], F32, kind="ExternalInput")
    gseld = nc.dram_tensor("gsel", [GROUPS, C], F32, kind="ExternalInput")
    yd = nc.dram_tensor("y", [HALF, C], F32, kind="ExternalOutput")

    xbt = xb[:, :].rearrange("(t p) c -> t p c", p=128)   # [32,128,512]
    xqt = xq[:, :].rearrange("(t p) c -> t p c", p=128)   # [16,128,512]
    yt = yd[:, :].rearrange("(t p) c -> t p c", p=128)    # [16,128,512]

    with tile.TileContext(nc) as tc:
        # ---------------- persistent storage + constants ----------------
        store = tc.alloc_tile_pool(name="store", bufs=1)
        kT = store.tile([128, CT, HW], F32R)      # kT[c%128, c//128, j]
        vS = store.tile([128, JT, C], BF16)       # v[j%128, j//128, c]
        qT = store.tile([128, CT, HALF], F32R)    # qT[c%128, c//128, i]
        wpr = store.tile([128, CT, C], F32R)      # wp rounded, [cin%128, cin//128, cout]
        cst = tc.alloc_tile_pool(name="cst", bufs=1)
        idn = cst.tile([128, 128], F32)
        nc.sync.dma_start(out=idn, in_=idnd[:, :])
        gsel = cst.tile([GROUPS, C], F32)
        nc.sync.dma_start(out=gsel, in_=gseld[:, :])
        onesf = cst.tile([128, 1], F32)
        nc.vector.memset(onesf, 1.0)
        onesb = cst.tile([128, 1], BF16)
        nc.vector.memset(onesb, 1.0)
        # f32r ones row for the rank-1 reciprocal broadcast (rounded producer)
        onesrow_f = cst.tile([1, 128], F32)
        nc.vector.memset(onesrow_f, 1.0)
        onesrow_r = cst.tile([1, 128], F32R)
        nc.vector.tensor_copy(onesrow_r[:, :], onesrow_f[:, :])
        bq_sb = cst.tile([128, CT], F32)
        bk_sb = cst.tile([128, CT], F32)
        gam_sb = cst.tile([128, CT], F32)
        bet_sb = cst.tile([128, CT], F32)
        for ct in range(CT):
            nc.sync.dma_start(out=bq_sb[:, ct:ct + 1], in_=bqd[ct * 128:(ct + 1) * 128, :])
            nc.sync.dma_start(out=bk_sb[:, ct:ct + 1], in_=bkd[ct * 128:(ct + 1) * 128, :])
            nc.sync.dma_start(out=gam_sb[:, ct:ct + 1], in_=gamd[ct * 128:(ct + 1) * 128, :])
            nc.sync.dma_start(out=bet_sb[:, ct:ct + 1], in_=betd[ct * 128:(ct + 1) * 128, :])
        bv_bc = cst.tile([128, C], F32)
        nc.sync.dma_start(out=bv_bc, in_=bvd[:].partition_broadcast(128))
        bp_bc = cst.tile([128, C], F32)
        nc.sync.dma_start(out=bp_bc, in_=bpd[:].partition_broadcast(128))
        s_sb = cst.tile([128, CT], F32)   # GN scale per channel
        t_sb = cst.tile([128, CT], F32)   # GN shift per channel

        # rounded weights for q/k/v (freed after phase B)
        wstage = tc.alloc_tile_pool(name="wstage", bufs=1)
        wrnd = tc.alloc_tile_pool(name="wrnd", bufs=1)
        wqr = wrnd.tile([128, CT, C], F32R)
        wkr = wrnd.tile([128, CT, C], F32R)
        wvr = wrnd.tile([128, CT, C], F32R)
        for wd, wr in ((wq, wqr), (wk, wkr), (wv, wvr), (wp, wpr)):
            stg = wstage.tile([128, CT, C], F32, tag="wstage")
            nc.sync.dma_start(
                out=stg, in_=wd[:, :].rearrange("(t p) c -> p t c", p=128))
            nc.vector.tensor_copy(wr[:, :, :], stg[:, :, :])

        # ---------------- phase A: GroupNorm statistics ----------------
        with tc.tile_pool(name="pa_sb", bufs=3) as pa, \
             tc.tile_pool(name="pa_ps", bufs=1, space="PSUM") as pa_ps, \
             tc.tile_pool(name="pa_small", bufs=1) as pas:
            psums = pa_ps.tile([GROUPS, 1], F32, tag="sums")
            psumsq = pa_ps.tile([GROUPS, 1], F32, tag="sumsq")
            for t in range(JT):
                xt = pa.tile([128, C], F32, tag="xa")
                nc.sync.dma_start(out=xt, in_=xbt[t, :, :])
                sq = pa.tile([128, C], F32, tag="sq")
                nc.scalar.activation(sq[:, :], xt[:, :], AF.Square)
                sp = pa.tile([128, 2 * GROUPS], F32, tag="sp")
                nc.vector.tensor_reduce(
                    sp[:, 0:GROUPS],
                    xt[:, :].rearrange("p (g d) -> p g d", g=GROUPS),
                    axis=AX.X, op=OP.add)
                nc.vector.tensor_reduce(
                    sp[:, GROUPS:2 * GROUPS],
                    sq[:, :].rearrange("p (g d) -> p g d", g=GROUPS),
                    axis=AX.X, op=OP.add)
                nc.tensor.matmul(psums[:, :], sp[:, 0:GROUPS], onesf[:, :],
                                 start=(t == 0), stop=(t == JT - 1))
                nc.tensor.matmul(psumsq[:, :], sp[:, GROUPS:2 * GROUPS], onesf[:, :],
                                 start=(t == 0), stop=(t == JT - 1))
            inv_n = 1.0 / float(HW * GSIZE)
            mean = pas.tile([GROUPS, 1], F32)
            nc.scalar.activation(mean[:, :], psums[:, :], AF.Copy, scale=inv_n)
            ex2 = pas.tile([GROUPS, 1], F32)
            nc.scalar.activation(ex2[:, :], psumsq[:, :], AF.Copy, scale=inv_n)
            var = pas.tile([GROUPS, 1], F32)
            nc.vector.tensor_mul(var[:, :], mean[:, :], mean[:, :])
            nc.vector.tensor_sub(var[:, :], ex2[:, :], var[:, :])
            epst = pas.tile([GROUPS, 1], F32)
            nc.vector.memset(epst, EPS)
            sd = pas.tile([GROUPS, 1], F32)
            nc.scalar.activation(sd[:, :], var[:, :], AF.Sqrt, bias=epst[:, :])
            mv2 = pas.tile([GROUPS, 2], F32)
            nc.vector.reciprocal(mv2[:, 1:2], sd[:, :])   # rstd
            nc.vector.tensor_copy(mv2[:, 0:1], mean[:, :])
            for ct in range(CT):
                pbc = pa_ps.tile([128, 2], F32, tag="bcast")
                nc.tensor.matmul(pbc[:, :], gsel[:, ct * 128:(ct + 1) * 128],
                                 mv2[:, :], start=True, stop=True)
                bc = pas.tile([128, 2], F32, tag="bc")
                nc.scalar.activation(bc[:, :], pbc[:, :], AF.Copy)
                nc.vector.tensor_mul(s_sb[:, ct:ct + 1], gam_sb[:, ct:ct + 1], bc[:, 1:2])
                tmp = pas.tile([128, 1], F32, tag="tmp")
                nc.vector.tensor_mul(tmp[:, :], bc[:, 0:1], s_sb[:, ct:ct + 1])
                nc.vector.tensor_sub(t_sb[:, ct:ct + 1], bet_sb[:, ct:ct + 1], tmp[:, :])

        # ---------------- phase B: transpose + normalize + K,V (and Q) ------
        def qkv_chunk(pb, pb_tp, pb_ps, src_tiled, jc, dst_is_q):
            """Process one 512-position chunk: transpose+normalize 4 x-tiles
            into hnT [c, 512], then the chunk's GEMMs."""
            hnT = pb.tile([128, CT, 512], F32R, tag="hnT")
            for jt in range(4):
                xt = pb.tile([128, C], F32, tag="xb")
                nc.sync.dma_start(out=xt, in_=src_tiled[jc * 4 + jt, :, :])
                for ct in range(CT):
                    tp = pb_tp.tile([128, 128], F32, tag="tp")
                    nc.tensor.transpose(tp[:, :], xt[:, ct * 128:(ct + 1) * 128], idn[:, :])
                    # hnT = s * xT + t   (per-channel, channels on partitions)
                    nc.vector.tensor_scalar(
                        hnT[:, ct, jt * 128:(jt + 1) * 128], tp[:, :],
                        s_sb[:, ct:ct + 1], t_sb[:, ct:ct + 1], OP.mult, OP.add)
            if dst_is_q:
                # qT[:, ct, jc*512...] = (wq^T @ hnT) + bq
                for ct in range(CT):
                    pq = pb_ps.tile([128, 512], F32, tag="qkv")
                    for k in range(CT):
                        nc.tensor.matmul(
                            pq[:, :], wqr[:, k, ct * 128:(ct + 1) * 128],
                            hnT[:, k, :], start=(k == 0), stop=(k == CT - 1))
                    nc.scalar.activation(
                        qT[:, ct, jc * 512:(jc + 1) * 512], pq[:, :],
                        AF.Identity, bias=bq_sb[:, ct:ct + 1])
            else:
                for ct in range(CT):
                    pk = pb_ps.tile([128, 512], F32, tag="qkv")
                    for k in range(CT):
                        nc.tensor.matmul(
                            pk[:, :], wkr[:, k, ct * 128:(ct + 1) * 128],
                            hnT[:, k, :], start=(k == 0), stop=(k == CT - 1))
                    nc.scalar.activation(
                        kT[:, ct, jc * 512:(jc + 1) * 512], pk[:, :],
                        AF.Identity, bias=bk_sb[:, ct:ct + 1])
                for jp in range(4):
                    pv = pb_ps.tile([128, 512], F32, tag="qkv")
                    for k in range(CT):
                        nc.tensor.matmul(
                            pv[:, :], hnT[:, k, jp * 128:(jp + 1) * 128],
                            wvr[:, k, :], start=(k == 0), stop=(k == CT - 1))
                    nc.vector.tensor_tensor(
                        vS[:, jc * 4 + jp, :], pv[:, :], bv_bc[:, :], OP.add)

        with tc.tile_pool(name="pb_sb", bufs=2) as pb, \
             tc.tile_pool(name="pb_tp", bufs=3, space="PSUM") as pb_tp, \
             tc.tile_pool(name="pb_ps", bufs=4, space="PSUM") as pb_ps:
            for jc in range(JC):
                qkv_chunk(pb, pb_tp, pb_ps, xbt, jc, dst_is_q=False)
            for jc in range(QC):
                qkv_chunk(pb, pb_tp, pb_ps, xqt, jc, dst_is_q=True)

        wrnd.release()    # free wq/wk/wv rounded copies (LIFO with wstage)
        wstage.release()

        # ---------------- phase C: attention + projection + residual --------
        with tc.tile_pool(name="pc_sb", bufs=3) as pcs, \
             tc.tile_pool(name="pc_o", bufs=2) as pco, \
             tc.tile_pool(name="ps_s", bufs=2, space="PSUM") as ps_s, \
             tc.tile_pool(name="ps_o", bufs=1, space="PSUM") as ps_o, \
             tc.tile_pool(name="ps_r", bufs=1, space="PSUM") as ps_r, \
             tc.tile_pool(name="ps_y", bufs=1, space="PSUM") as ps_y:
            for ib in range(IB):
                po = ps_o.tile([128, CT, 512], F32)
                pr = ps_r.tile([1, 512], F32)
                for j in range(JT):
                    pss = ps_s.tile([128, 512], F32, tag="scores")
                    for k in range(CT):
                        nc.tensor.matmul(
                            pss[:, :], kT[:, k, j * 128:(j + 1) * 128],
                            qT[:, k, ib * 512:(ib + 1) * 512],
                            start=(k == 0), stop=(k == CT - 1))
                    et = pcs.tile([128, 512], BF16, tag="exp")
                    nc.scalar.activation(et[:, :], pss[:, :], AF.Exp, scale=SM_SCALE)
                    for ct in range(CT):
                        nc.tensor.matmul(
                            po[:, ct, :], vS[:, j, ct * 128:(ct + 1) * 128],
                            et[:, :], start=(j == 0), stop=(j == JT - 1))
                    # row-sums of exp: ones^T @ expT -> [1, 512] (i on free dim)
                    nc.tensor.matmul(
                        pr[:, :], onesb[:, :], et[:, :],
                        start=(j == 0), stop=(j == JT - 1))
                # reciprocal of row sums, then rank-1 broadcast to [128, 512]
                rrow_f = pcs.tile([1, 512], F32, tag="rrowf")
                nc.vector.reciprocal(rrow_f[:, :], pr[:, :])
                rrow = pcs.tile([1, 512], F32R, tag="rrow")
                nc.vector.tensor_copy(rrow[:, :], rrow_f[:, :])
                pbc = ps_y.tile([128, 512], F32, tag="proj")
                nc.tensor.matmul(pbc[:, :], onesrow_r[:, :], rrow[:, :],
                                 start=True, stop=True)
                rbc = pcs.tile([128, 512], F32, tag="rbc")
                nc.scalar.activation(rbc[:, :], pbc[:, :], AF.Copy)
                ot = pco.tile([128, CT, 512], F32R, tag="outT")
                for ct in range(CT):
                    nc.vector.tensor_tensor(
                        ot[:, ct, :], po[:, ct, :], rbc[:, :], OP.mult)
                for ip in range(4):
                    py = ps_y.tile([128, 512], F32, tag="proj")
                    for ct in range(CT):
                        nc.tensor.matmul(
                            py[:, :], ot[:, ct, ip * 128:(ip + 1) * 128],
                            wpr[:, ct, :], start=(ct == 0), stop=(ct == CT - 1))
                    xr = pcs.tile([128, C], F32, tag="xr")
                    nc.sync.dma_start(out=xr, in_=xqt[ib * 4 + ip, :, :])
                    bpx = pcs.tile([128, C], F32, tag="bpx")
                    nc.vector.tensor_tensor(bpx[:, :], xr[:, :], bp_bc[:, :], OP.add)
                    y2 = pcs.tile([128, C], F32, tag="y2")
                    nc.vector.tensor_tensor(y2[:, :], py[:, :], bpx[:, :], OP.add)
                    nc.sync.dma_start(out=yt[ib * 4 + ip, :, :], in_=y2[:, :])

        cst.release()
        store.release()

    _split_multi_waits(nc)
    return nc


_PROGRAM = None


def _get_program():
    global _PROGRAM
    if _PROGRAM is None:
        _PROGRAM = build_program()
    return _PROGRAM


def make_in_maps(x, gamma, beta, wq, bq, wk, bk, wv, bv, wp, bp):
    f32 = lambda a: np.ascontiguousarray(a, dtype=np.float32)
    xr = f32(x).reshape(B, HW, C)
    gsel = np.zeros((GROUPS, C), dtype=np.float32)
    for g in range(GROUPS):
        gsel[g, g * GSIZE:(g + 1) * GSIZE] = 1.0
    common = {
        "wq": f32(wq), "wk": f32(wk), "wv": f32(wv), "wp": f32(wp),
        "bq": f32(bq).reshape(C, 1), "bk": f32(bk).reshape(C, 1),
        "bv": f32(bv), "bp": f32(bp),
        "gamma": f32(gamma).reshape(C, 1), "beta": f32(beta).reshape(C, 1),
        "idn": np.eye(128, dtype=np.float32), "gsel": gsel,
    }
    in_maps = []
    for c in range(N_CORES):
        b, h = c // 2, c % 2
        m = dict(common)
        m["xb"] = xr[b]
        m["xq"] = np.ascontiguousarray(xr[b, h * HALF:(h + 1) * HALF])
        in_maps.append(m)
    return in_maps


def kernel(x, gamma, beta, wq, bq, wk, bk, wv, bv, wp, bp, _trace=False):
    nc = _get_program()
    in_maps = make_in_maps(x, gamma, beta, wq, bq, wk, bk, wv, bv, wp, bp)
    res = run_bass_kernel_spmd(nc, in_maps, list(range(N_CORES)), trace=_trace)
    out = np.empty((B, HW, C), dtype=np.float32)
    for c in range(N_CORES):
        b, h = c // 2, c % 2
        out[b, h * HALF:(h + 1) * HALF] = res.results[c]["y"]
    if _trace:
        kernel._last_result = res
    return out.reshape(B, H, W, C)


# revision 10
# speedup vs baseline: 1.3080x; 1.3080x over previous
"""AttentionBlock (GroupNorm + single-head 4096x4096 attention + proj + residual)
on 8 Trainium2 NeuronCores.

Sharding: core c = 2*b + h handles image b (of 4), query-half h (of 2).
Each core:
  - computes GroupNorm stats for its image (all 4096 positions),
  - computes kT [512,4096] and v [4096,512] for the full image,
  - computes qT for its 2048 query rows,
  - flash-style attention over its 2048 queries (softmax without max
    subtraction -- scores are O(6) so exp is safe in fp32),
  - projection + bias + residual for its rows.
No collectives; k/v compute is duplicated across the half-pair (~10% FLOPs).

Precision: float32r (full-speed PE, ~1e-4 rel err) for transposes, QKV,
scores and projection; bf16 for the exp(P)@V contraction; fp32 for GN stats,
softmax normalization and the residual add.
"""

import sys

sys.path.insert(0, "/opt/trn_rl_repo")

import numpy as np  # noqa: E402

import bass_rust  # noqa: E402
import concourse.bass as bass  # noqa: E402
import concourse.mybir as mybir  # noqa: E402
import concourse.tile as tile  # noqa: E402
from concourse.vector_clock import ScopedClock  # noqa: E402
from concourse.bass_utils import run_bass_kernel_spmd  # noqa: E402

F32 = mybir.dt.float32
F32R = mybir.dt.float32r
BF16 = mybir.dt.bfloat16
AF = mybir.ActivationFunctionType
OP = mybir.AluOpType
AX = mybir.AxisListType

B, H, W, C = 4, 64, 64, 512
HW = H * W            # 4096 positions per image
HALF = HW // 2        # 2048 query rows per core
GROUPS = 32
GSIZE = C // GROUPS   # 16 channels per group
EPS = 1e-5
SM_SCALE = 1.0 / float(np.sqrt(C))
N_CORES = 8
CT = C // 128         # 4 channel partition-tiles
JT = HW // 128        # 32 position partition-tiles
JC = HW // 512        # 8 position chunks (kT/v build)
QC = HALF // 512      # 4 query chunks (qT build)
IB = HALF // 512      # 4 query i-blocks (attention)


# --- workaround: walrus in this container rejects instructions carrying more
# than one sync-wait command.  Split extra waits onto same-engine NOPs placed
# immediately before the instruction (engine program order makes this exact).
def _split_multi_waits(nc, max_waits=1):
    n = 0
    for f in nc.m.functions:
        for bb in f.blocks:
            newlist = []
            for inst in bb.instructions:
                si = inst.sync_info
                waits = list(si.on_wait) if si is not None else []
                if len(waits) > max_waits:
                    n += 1
                    for k, wt in enumerate(waits[:-max_waits]):
                        nop = bass_rust.InstNoOp(
                            name=f"{inst.name}-sw{k}", engine=inst.engine)
                        nop.sync_info = mybir.SyncInfo(on_wait=[wt], on_update=[])
                        newlist.append(nop)
                    inst.sync_info = mybir.SyncInfo(
                        on_wait=waits[-max_waits:], on_update=list(si.on_update))
                newlist.append(inst)
            bb.instructions[:] = newlist
    return n


def _split_drain_and_barrier(self, tick_clock, wait_clock):
    # same as TileContext._drain_and_barrier but with the tail drain's waits
    # split onto single-wait NOPs (same walrus limitation as above).
    drain_inst = self.nc.sync.drain()
    wait_clock.add_sem_waits(
        drain_inst.ins, ScopedClock({None: tick_clock.global_clock}))
    mi = drain_inst.ins
    waits = list(mi.sync_info.on_wait) if mi.sync_info is not None else []
    if len(waits) > 1:
        mi.sync_info.on_wait = []
        for wt in waits:
            wi = self.nc.sync.nop(nofuse=True, hint="tail_drain_wait")
            wi.ins.sync_info = mybir.SyncInfo(on_wait=[wt], on_update=[])
    self.nc.all_engine_barrier()
    assert self.sems is not None
    popped = self.nc._tile_sem_poison_stack.pop()
    assert popped is self._sem_poison
    self.nc.clear_and_free_semaphores(list(self.sems.allocated().values()))
    self.nc.all_engine_barrier()


tile.TileContext._drain_and_barrier = _split_drain_and_barrier


def build_program():
    nc = bass.Bass()

    xb = nc.dram_tensor("xb", [HW, C], F32, kind="ExternalInput")
    xq = nc.dram_tensor("xq", [HALF, C], F32, kind="ExternalInput")
    wq = nc.dram_tensor("wq", [C, C], F32, kind="ExternalInput")
    wk = nc.dram_tensor("wk", [C, C], F32, kind="ExternalInput")
    wv = nc.dram_tensor("wv", [C, C], F32, kind="ExternalInput")
    wp = nc.dram_tensor("wp", [C, C], F32, kind="ExternalInput")
    bqd = nc.dram_tensor("bq", [C, 1], F32, kind="ExternalInput")
    bkd = nc.dram_tensor("bk", [C, 1], F32, kind="ExternalInput")
    bvd = nc.dram_tensor("bv", [C], F32, kind="ExternalInput")
    bpd = nc.dram_tensor("bp", [C], F32, kind="ExternalInput")
    gamd = nc.dram_tensor("gamma", [C, 1], F32, kind="ExternalInput")
    betd = nc.dram_tensor("beta", [C, 1], F32, kind="ExternalInput")
    idnd = nc.dram_tensor("idn", [128, 128], F32, kind="ExternalInput")
    gseld = nc.dram_tensor("gsel", [GROUPS, C], F32, kind="ExternalInput")
    yd = nc.dram_tensor("y", [HALF, C], F32, kind="ExternalOutput")

    xbt = xb[:, :].rearrange("(t p) c -> t p c", p=128)   # [32,128,512]
    xqt = xq[:, :].rearrange("(t p) c -> t p c", p=128)   # [16,128,512]
    yt = yd[:, :].rearrange("(t p) c -> t p c", p=128)    # [16,128,512]

    with tile.TileContext(nc) as tc:
        # ---------------- persistent storage + constants ----------------
        store = tc.alloc_tile_pool(name="store", bufs=1)
        kT = store.tile([128, CT, HW], F32R)      # kT[c%128, c//128, j]
        vS = store.tile([128, JT, C], BF16)       # v[j%128, j//128, c]
        qT = store.tile([128, CT, HALF], F32R)    # qT[c%128, c//128, i]
        wpr = store.tile([128, CT, C], F32R)      # wp rounded, [cin%128, cin//128, cout]
        cst = tc.alloc_tile_pool(name="cst", bufs=1)
        idn = cst.tile([128, 128], F32)
        nc.sync.dma_start(out=idn, in_=idnd[:, :])
        gsel = cst.tile([GROUPS, C], F32)
        nc.sync.dma_start(out=gsel, in_=gseld[:, :])
        onesf = cst.tile([128, 1], F32)
        nc.vector.memset(onesf, 1.0)
        onesb = cst.tile([128, 1], BF16)
        nc.vector.memset(onesb, 1.0)
        # f32r ones row for the rank-1 reciprocal broadcast (rounded producer)
        onesrow_f = cst.tile([1, 128], F32)
        nc.vector.memset(onesrow_f, 1.0)
        onesrow_r = cst.tile([1, 128], F32R)
        nc.vector.tensor_copy(onesrow_r[:, :], onesrow_f[:, :])
        bq_sb = cst.tile([128, CT], F32)
        bk_sb = cst.tile([128, CT], F32)
        gam_sb = cst.tile([128, CT], F32)
        bet_sb = cst.tile([128, CT], F32)
        for ct in range(CT):
            nc.sync.dma_start(out=bq_sb[:, ct:ct + 1], in_=bqd[ct * 128:(ct + 1) * 128, :])
            nc.sync.dma_start(out=bk_sb[:, ct:ct + 1], in_=bkd[ct * 128:(ct + 1) * 128, :])
            nc.sync.dma_start(out=gam_sb[:, ct:ct + 1], in_=gamd[ct * 128:(ct + 1) * 128, :])
            nc.sync.dma_start(out=bet_sb[:, ct:ct + 1], in_=betd[ct * 128:(ct + 1) * 128, :])
        bv_bc = cst.tile([128, C], F32)
        nc.sync.dma_start(out=bv_bc, in_=bvd[:].partition_broadcast(128))
        bp_bc = cst.tile([128, C], F32)
        nc.sync.dma_start(out=bp_bc, in_=bpd[:].partition_broadcast(128))
        s_sb = cst.tile([128, CT], F32)   # GN scale per channel
        t_sb = cst.tile([128, CT], F32)   # GN shift per channel

        # rounded weights for q/k/v (freed after phase B)
        wstage = tc.alloc_tile_pool(name="wstage", bufs=1)
        wrnd = tc.alloc_tile_pool(name="wrnd", bufs=1)
        wqr = wrnd.tile([128, CT, C], F32R)
        wkr = wrnd.tile([128, CT, C], F32R)
        wvr = wrnd.tile([128, CT, C], F32R)
        for wd, wr in ((wq, wqr), (wk, wkr), (wv, wvr), (wp, wpr)):
            stg = wstage.tile([128, CT, C], F32, tag="wstage")
            nc.sync.dma_start(
                out=stg, in_=wd[:, :].rearrange("(t p) c -> p t c", p=128))
            nc.vector.tensor_copy(wr[:, :, :], stg[:, :, :])

        # ---------------- phase A: GroupNorm statistics ----------------
        with tc.tile_pool(name="pa_sb", bufs=3) as pa, \
             tc.tile_pool(name="pa_ps", bufs=1, space="PSUM") as pa_ps, \
             tc.tile_pool(name="pa_small", bufs=1) as pas:
            psums = pa_ps.tile([GROUPS, 1], F32, tag="sums")
            psumsq = pa_ps.tile([GROUPS, 1], F32, tag="sumsq")
            for t in range(JT):
                xt = pa.tile([128, C], F32, tag="xa")
                nc.sync.dma_start(out=xt, in_=xbt[t, :, :])
                sq = pa.tile([128, C], F32, tag="sq")
                nc.scalar.activation(sq[:, :], xt[:, :], AF.Square)
                sp = pa.tile([128, 2 * GROUPS], F32, tag="sp")
                nc.vector.tensor_reduce(
                    sp[:, 0:GROUPS],
                    xt[:, :].rearrange("p (g d) -> p g d", g=GROUPS),
                    axis=AX.X, op=OP.add)
                nc.vector.tensor_reduce(
                    sp[:, GROUPS:2 * GROUPS],
                    sq[:, :].rearrange("p (g d) -> p g d", g=GROUPS),
                    axis=AX.X, op=OP.add)
                nc.tensor.matmul(psums[:, :], sp[:, 0:GROUPS], onesf[:, :],
                                 start=(t == 0), stop=(t == JT - 1))
                nc.tensor.matmul(psumsq[:, :], sp[:, GROUPS:2 * GROUPS], onesf[:, :],
                                 start=(t == 0), stop=(t == JT - 1))
            inv_n = 1.0 / float(HW * GSIZE)
            mean = pas.tile([GROUPS, 1], F32)
            nc.scalar.activation(mean[:, :], psums[:, :], AF.Copy, scale=inv_n)
            ex2 = pas.tile([GROUPS, 1], F32)
            nc.scalar.activation(ex2[:, :], psumsq[:, :], AF.Copy, scale=inv_n)
            var = pas.tile([GROUPS, 1], F32)
            nc.vector.tensor_mul(var[:, :], mean[:, :], mean[:, :])
            nc.vector.tensor_sub(var[:, :], ex2[:, :], var[:, :])
            epst = pas.tile([GROUPS, 1], F32)
            nc.vector.memset(epst, EPS)
            sd = pas.tile([GROUPS, 1], F32)
            nc.scalar.activation(sd[:, :], var[:, :], AF.Sqrt, bias=epst[:, :])
            mv2 = pas.tile([GROUPS, 2], F32)
            nc.vector.reciprocal(mv2[:, 1:2], sd[:, :])   # rstd
            nc.vector.tensor_copy(mv2[:, 0:1], mean[:, :])
            for ct in range(CT):
                pbc = pa_ps.tile([128, 2], F32, tag="bcast")
                nc.tensor.matmul(pbc[:, :], gsel[:, ct * 128:(ct + 1) * 128],
                                 mv2[:, :], start=True, stop=True)
                bc = pas.tile([128, 2], F32, tag="bc")
                nc.scalar.activation(bc[:, :], pbc[:, :], AF.Copy)
                nc.vector.tensor_mul(s_sb[:, ct:ct + 1], gam_sb[:, ct:ct + 1], bc[:, 1:2])
                tmp = pas.tile([128, 1], F32, tag="tmp")
                nc.vector.tensor_mul(tmp[:, :], bc[:, 0:1], s_sb[:, ct:ct + 1])
                nc.vector.tensor_sub(t_sb[:, ct:ct + 1], bet_sb[:, ct:ct + 1], tmp[:, :])

        # ---------------- phase B: transpose + normalize + K,V (and Q) ------
        def qkv_chunk(pb, pb_tp, pb_ps, src_tiled, jc, dst_is_q):
            """Process one 512-position chunk: transpose+normalize 4 x-tiles
            into hnT [c, 512], then the chunk's GEMMs."""
            hnT = pb.tile([128, CT, 512], F32R, tag="hnT")
            xts = []
            for jt in range(4):
                xt = pb.tile([128, C], F32, tag=f"xb{jt}")
                nc.sync.dma_start(out=xt, in_=src_tiled[jc * 4 + jt, :, :])
                xts.append(xt)
            for ct in range(CT):
                # 4 transposes of this channel-tile land in quarters of one
                # PSUM bank; one DVE op normalizes the whole 512-pos chunk.
                tp = pb_tp.tile([128, 4, 128], F32, tag="tp")
                for jt in range(4):
                    nc.tensor.transpose(
                        tp[:, jt, :], xts[jt][:, ct * 128:(ct + 1) * 128], idn[:, :])
                # hnT = s * xT + t   (per-channel, channels on partitions)
                nc.vector.tensor_scalar(
                    hnT[:, ct, :], tp[:, :, :].rearrange("p a b -> p (a b)"),
                    s_sb[:, ct:ct + 1], t_sb[:, ct:ct + 1], OP.mult, OP.add)
            if dst_is_q:
                # qT[:, ct, jc*512...] = (wq^T @ hnT) + bq
                for ct in range(CT):
                    pq = pb_ps.tile([128, 512], F32, tag="qkv")
                    for k in range(CT):
                        nc.tensor.matmul(
                            pq[:, :], wqr[:, k, ct * 128:(ct + 1) * 128],
                            hnT[:, k, :], start=(k == 0), stop=(k == CT - 1))
                    nc.scalar.activation(
                        qT[:, ct, jc * 512:(jc + 1) * 512], pq[:, :],
                        AF.Identity, bias=bq_sb[:, ct:ct + 1])
            else:
                for ct in range(CT):
                    pk = pb_ps.tile([128, 512], F32, tag="qkv")
                    for k in range(CT):
                        nc.tensor.matmul(
                            pk[:, :], wkr[:, k, ct * 128:(ct + 1) * 128],
                            hnT[:, k, :], start=(k == 0), stop=(k == CT - 1))
                    nc.scalar.activation(
                        kT[:, ct, jc * 512:(jc + 1) * 512], pk[:, :],
                        AF.Identity, bias=bk_sb[:, ct:ct + 1])
                for jp in range(4):
                    pv = pb_ps.tile([128, 512], F32, tag="qkv")
                    for k in range(CT):
                        nc.tensor.matmul(
                            pv[:, :], hnT[:, k, jp * 128:(jp + 1) * 128],
                            wvr[:, k, :], start=(k == 0), stop=(k == CT - 1))
                    nc.vector.tensor_tensor(
                        vS[:, jc * 4 + jp, :], pv[:, :], bv_bc[:, :], OP.add)

        with tc.tile_pool(name="pb_sb", bufs=2) as pb, \
             tc.tile_pool(name="pb_tp", bufs=3, space="PSUM") as pb_tp, \
             tc.tile_pool(name="pb_ps", bufs=4, space="PSUM") as pb_ps:
            for jc in range(JC):
                qkv_chunk(pb, pb_tp, pb_ps, xbt, jc, dst_is_q=False)
            for jc in range(QC):
                qkv_chunk(pb, pb_tp, pb_ps, xqt, jc, dst_is_q=True)

        wrnd.release()    # free wq/wk/wv rounded copies (LIFO with wstage)
        wstage.release()

        # ---------------- phase C: attention + projection + residual --------
        with tc.tile_pool(name="pc_sb", bufs=3) as pcs, \
             tc.tile_pool(name="pc_o", bufs=2) as pco, \
             tc.tile_pool(name="ps_s", bufs=2, space="PSUM") as ps_s, \
             tc.tile_pool(name="ps_o", bufs=1, space="PSUM") as ps_o, \
             tc.tile_pool(name="ps_r", bufs=1, space="PSUM") as ps_r, \
             tc.tile_pool(name="ps_y", bufs=1, space="PSUM") as ps_y:
            for ib in range(IB):
                po = ps_o.tile([128, CT, 512], F32)
                pr = ps_r.tile([1, 512], F32)
                for j in range(JT):
                    pss = ps_s.tile([128, 512], F32, tag="scores")
                    for k in range(CT):
                        nc.tensor.matmul(
                            pss[:, :], kT[:, k, j * 128:(j + 1) * 128],
                            qT[:, k, ib * 512:(ib + 1) * 512],
                            start=(k == 0), stop=(k == CT - 1))
                    et = pcs.tile([128, 512], BF16, tag="exp")
                    nc.scalar.activation(et[:, :], pss[:, :], AF.Exp, scale=SM_SCALE)
                    for ct in range(CT):
                        nc.tensor.matmul(
                            po[:, ct, :], vS[:, j, ct * 128:(ct + 1) * 128],
                            et[:, :], start=(j == 0), stop=(j == JT - 1))
                    # row-sums of exp: ones^T @ expT -> [1, 512] (i on free dim)
                    nc.tensor.matmul(
                        pr[:, :], onesb[:, :], et[:, :],
                        start=(j == 0), stop=(j == JT - 1))
                # rank-1 broadcast of the row sums to [128, 512], then a
                # full-width reciprocal (the [1,512] form is lane-starved)
                srow = pcs.tile([1, 512], F32R, tag="srow")
                nc.scalar.activation(srow[:, :], pr[:, :], AF.Identity)
                pbc = ps_y.tile([128, 512], F32, tag="proj")
                nc.tensor.matmul(pbc[:, :], onesrow_r[:, :], srow[:, :],
                                 start=True, stop=True)
                rbc = pcs.tile([128, 512], F32, tag="rbc")
                nc.vector.reciprocal(rbc[:, :], pbc[:, :])
                ot = pco.tile([128, CT, 512], F32R, tag="outT")
                for ct in range(CT):
                    nc.vector.tensor_tensor(
                        ot[:, ct, :], po[:, ct, :], rbc[:, :], OP.mult)
                for ip in range(4):
                    py = ps_y.tile([128, 512], F32, tag="proj")
                    for ct in range(CT):
                        nc.tensor.matmul(
                            py[:, :], ot[:, ct, ip * 128:(ip + 1) * 128],
                            wpr[:, ct, :], start=(ct == 0), stop=(ct == CT - 1))
                    xr = pcs.tile([128, C], F32, tag="xr")
                    nc.sync.dma_start(out=xr, in_=xqt[ib * 4 + ip, :, :])
                    bpx = pcs.tile([128, C], F32, tag="bpx")
                    nc.vector.tensor_tensor(bpx[:, :], xr[:, :], bp_bc[:, :], OP.add)
                    y2 = pcs.tile([128, C], F32, tag="y2")
                    nc.vector.tensor_tensor(y2[:, :], py[:, :], bpx[:, :], OP.add)
                    nc.sync.dma_start(out=yt[ib * 4 + ip, :, :], in_=y2[:, :])

        cst.release()
        store.release()

    _split_multi_waits(nc)
    return nc


_PROGRAM = None


def _get_program():
    global _PROGRAM
    if _PROGRAM is None:
        _PROGRAM = build_program()
    return _PROGRAM


def make_in_maps(x, gamma, beta, wq, bq, wk, bk, wv, bv, wp, bp):
    f32 = lambda a: np.ascontiguousarray(a, dtype=np.float32)
    xr = f32(x).reshape(B, HW, C)
    gsel = np.zeros((GROUPS, C), dtype=np.float32)
    for g in range(GROUPS):
        gsel[g, g * GSIZE:(g + 1) * GSIZE] = 1.0
    common = {
        "wq": f32(wq), "wk": f32(wk), "wv": f32(wv), "wp": f32(wp),
        "bq": f32(bq).reshape(C, 1), "bk": f32(bk).reshape(C, 1),
        "bv": f32(bv), "bp": f32(bp),
        "gamma": f32(gamma).reshape(C, 1), "beta": f32(beta).reshape(C, 1),
        "idn": np.eye(128, dtype=np.float32), "gsel": gsel,
    }
    in_maps = []
    for c in range(N_CORES):
        b, h = c // 2, c % 2
        m = dict(common)
        m["xb"] = xr[b]
        m["xq"] = np.ascontiguousarray(xr[b, h * HALF:(h + 1) * HALF])
        in_maps.append(m)
    return in_maps


def kernel(x, gamma, beta, wq, bq, wk, bk, wv, bv, wp, bp, _trace=False):
    nc = _get_program()
    in_maps = make_in_maps(x, gamma, beta, wq, bq, wk, bk, wv, bv, wp, bp)
    res = run_bass_kernel_spmd(nc, in_maps, list(range(N_CORES)), trace=_trace)
    out = np.empty((B, HW, C), dtype=np.float32)
    for c in range(N_CORES):
        b, h = c // 2, c % 2
        out[b, h * HALF:(h + 1) * HALF] = res.results[c]["y"]
    if _trace:
        kernel._last_result = res
    return out.reshape(B, H, W, C)
